# revision 25
# baseline (speedup 1.0000x reference)
"""PerformerAttention (softmax attention + interleaved RoPE) on 8 trn2 cores.

Sharding: data-parallel over batch (16 batches -> 2 per core), weights replicated.

v2 design (all-bf16, transposed attention):
  qk^T = wqk^T.T @ x^T (bf16, fp32 accum) -> RoPE -> bf16 qkT tiles
  S^T = k^T q per head ([keys, queries] layout), head PAIRS row-packed on the
    128-row PE array (K=64 each -> tile_position (0,0)/(64,0) run concurrently)
  P^T = exp(S^T)  -- no max subtraction (logits bounded ~|8.7| for this data)
  out^T/sums = [V | 1]^T @ P^T  -- ones column interleaved in V gives softmax
    row-sums as psum row 64 for free
  attnT = out^T * (1/sums) broadcast across partitions (gpsimd bcast + DVE mul)
  final = attnT.T @ wproj^T + bias

Host-side prep: shard x over batch, transpose + bf16-cast, permute wqkv q/k
rows for contiguous-block rope, fold 1/sqrt(D) into wq.
"""

import numpy as np
import ml_dtypes

import concourse.bass as bass
import concourse.mybir as mybir
import concourse.tile as tile
from concourse import bacc
from concourse.bass_utils import run_bass_kernel_spmd

F32 = mybir.dt.float32
BF16 = mybir.dt.bfloat16
COPY = mybir.ActivationFunctionType.Copy
EXP = mybir.ActivationFunctionType.Exp
LN = mybir.ActivationFunctionType.Ln

B, N, C, H, D = 16, 577, 768, 12, 64
NCORES = 8
BPC = B // NCORES  # batches per core
NM1 = N - 1  # 576 rope'd tokens
NT = [(0, 128), (128, 128), (256, 128), (384, 128), (512, 65)]  # token tiles
NCH = [(0, 512), (512, 65)]   # token free-dim chunks (psum bank = 512 fp32)
FCH = [(0, 512), (512, 256)]  # feature free-dim chunks
VW = 65 * H  # 780: V stored as 12 blocks of [v_h | ones]

_CACHED_NC = None
LAST_RESULTS = None  # test harness reads exec_time_ns off this


def _build_nc():
    nc = bacc.Bacc("TRN2", target_bir_lowering=False)

    xT_d = nc.dram_tensor("xT", [BPC, C, N], BF16, kind="ExternalInput")
    wqk_d = nc.dram_tensor("wqkT", [C, 2 * C], BF16, kind="ExternalInput")
    wv_d = nc.dram_tensor("wvT", [C, C], BF16, kind="ExternalInput")
    wp_d = nc.dram_tensor("wpT", [C, C], BF16, kind="ExternalInput")
    c_d = nc.dram_tensor("c128", [128, NM1], BF16, kind="ExternalInput")
    s_d = nc.dram_tensor("s128", [128, NM1], BF16, kind="ExternalInput")
    bias_d = nc.dram_tensor("biasb", [128, C], F32, kind="ExternalInput")
    out_d = nc.dram_tensor("out", [BPC, N, C], F32, kind="ExternalOutput")

    with tile.TileContext(nc) as tc:
        with (
            tc.tile_pool(name="const", bufs=1) as constp,
            tc.tile_pool(name="xp", bufs=2) as xp,
            tc.tile_pool(name="qkp", bufs=2) as qkp,
            tc.tile_pool(name="vp", bufs=2) as vp,
            tc.tile_pool(name="atp", bufs=2) as atp,
            tc.tile_pool(name="work", bufs=3) as work,
            tc.tile_pool(name="pep", bufs=2) as pep,
            tc.tile_pool(name="rp", bufs=2) as rp,
            tc.tile_pool(name="obp", bufs=3) as obp,
            tc.tile_pool(name="psA", bufs=4, space="PSUM") as psA,
            tc.tile_pool(name="psB", bufs=4, space="PSUM") as psB,
        ):
            # ---- constants / weights (once per core) ----
            wqk = []
            wv = []
            wp = []
            for ct in range(6):
                t = constp.tile([128, 2 * C], BF16, name=f"wqk{ct}", tag=f"wqk{ct}")
                nc.sync.dma_start(t, wqk_d[ct * 128:(ct + 1) * 128, :])
                wqk.append(t)
                t = constp.tile([128, C], BF16, name=f"wv{ct}", tag=f"wv{ct}")
                nc.sync.dma_start(t, wv_d[ct * 128:(ct + 1) * 128, :])
                wv.append(t)
                t = constp.tile([128, C], BF16, name=f"wp{ct}", tag=f"wp{ct}")
                nc.sync.dma_start(t, wp_d[ct * 128:(ct + 1) * 128, :])
                wp.append(t)
            c128 = constp.tile([128, NM1], BF16, name="c128", tag="c128")
            nc.sync.dma_start(c128, c_d[:, :])
            s128 = constp.tile([128, NM1], BF16, name="s128", tag="s128")
            nc.sync.dma_start(s128, s_d[:, :])
            biasb = constp.tile([128, C], F32, name="biasb", tag="biasb")
            nc.sync.dma_start(biasb, bias_d[:, :])

            for b in range(BPC):
                # recip-broadcast source window; lanes 1..31 of quadrant 2
                # must be initialized for stream_shuffle's [64:96] reads
                recs = rp.tile([128, N], F32, name="recs", tag="recs")
                nc.vector.memset(recs[64:96, :], 1.0)

                # ---- load x^T bf16 tiles ----
                xT = []
                for ct in range(6):
                    t = xp.tile([128, N], BF16, name=f"xT{ct}", tag=f"xT{ct}")
                    nc.sync.dma_start(t, xT_d[b, ct * 128:(ct + 1) * 128, :])
                    xT.append(t)

                # ---- qk^T = wqk^T.T @ x^T  [f, n], then RoPE -> bf16 ----
                qkT = []
                for ft in range(12):
                    psq = []
                    for (cs, cw) in NCH:
                        ps = psA.tile([128, 512], F32, name="ps_qk", tag="psA")
                        for ct in range(6):
                            nc.tensor.matmul(
                                ps[:, 0:cw],
                                lhsT=wqk[ct][:, ft * 128:(ft + 1) * 128],
                                rhs=xT[ct][:, cs:cs + cw],
                                start=(ct == 0), stop=(ct == 5),
                            )
                        psq.append(ps)
                    wkt = work.tile([128, N], BF16, name="wkt", tag="wkt")
                    nc.scalar.activation(wkt[:, 0:512], psq[0][:, 0:512], COPY,
                                         bias=0.0, scale=1.0)
                    nc.scalar.activation(wkt[:, 512:N], psq[1][:, 0:65], COPY,
                                         bias=0.0, scale=1.0)
                    qk = qkp.tile([128, N], BF16, name=f"qkT{ft}",
                                  tag=f"qkT{ft}")
                    tsw = work.tile([128, NM1], BF16, name="tsw", tag="tsw")
                    rot = work.tile([128, NM1], BF16, name="rot", tag="rot")
                    # tsw = [to0; te0; to1; te1] (swap 32-row even/odd blocks)
                    nc.vector.tensor_copy(tsw[0:32, :], wkt[32:64, 1:N])
                    nc.vector.tensor_copy(tsw[32:64, :], wkt[0:32, 1:N])
                    nc.vector.tensor_copy(tsw[64:96, :], wkt[96:128, 1:N])
                    nc.vector.tensor_copy(tsw[96:128, :], wkt[64:96, 1:N])
                    nc.vector.tensor_mul(rot, wkt[:, 1:N], c128)
                    nc.vector.tensor_mul(tsw, tsw, s128)  # s128 = [-s;s;-s;s]
                    nc.vector.tensor_add(qk[:, 1:N], rot, tsw)
                    nc.vector.tensor_copy(qk[:, 0:1], wkt[:, 0:1])  # CLS
                    qkT.append(qk)

                # ---- V = x @ wv^T, interleaved [v_h | 1] blocks of 65 ----
                V = []
                for (ns, nsz) in NT:
                    nt = ns // 128
                    vt = vp.tile([128, VW], BF16, name=f"V{nt}", tag=f"V{nt}")
                    ones_ap = vt.rearrange("p (h d) -> p h d", d=65)[:, :, 64:65]
                    nc.vector.memset(ones_ap, 1.0)
                    psv = []
                    for (fs, fw) in FCH:
                        ps = psB.tile([128, 512], F32, name="ps_v", tag="psB")
                        for ct in range(6):
                            nc.tensor.matmul(
                                ps[0:nsz, 0:fw],
                                lhsT=xT[ct][:, ns:ns + nsz],
                                rhs=wv[ct][:, fs:fs + fw],
                                start=(ct == 0), stop=(ct == 5),
                            )
                        psv.append(ps)
                    dst0 = vt[0:nsz, 0:520].rearrange(
                        "p (h d) -> p h d", d=65)[:, :, 0:64]
                    src0 = psv[0][0:nsz, 0:512].rearrange(
                        "p (h d) -> p h d", d=64)
                    nc.vector.tensor_copy(dst0, src0)
                    dst1 = vt[0:nsz, 520:780].rearrange(
                        "p (h d) -> p h d", d=65)[:, :, 0:64]
                    src1 = psv[1][0:nsz, 0:256].rearrange(
                        "p (h d) -> p h d", d=64)
                    nc.vector.tensor_copy(dst1, src1)
                    V.append(vt)

                # ---- attention, head pairs (row-packed S^T matmuls) ----
                ATT = []
                for hp in range(6):
                    at = atp.tile([128, N], BF16, name=f"attnT{hp}",
                                  tag=f"attnT{hp}")
                    ATT.append(at)
                    Pe = {0: [], 1: []}
                    for (ms, msz) in NT:
                        psS = {0: [], 1: []}
                        for ci, (cs, cw) in enumerate(NCH):
                            for hi in range(2):
                                h = 2 * hp + hi
                                qt = qkT[h // 2][hi * 64:hi * 64 + 64, :]
                                kt = qkT[6 + h // 2][hi * 64:hi * 64 + 64, :]
                                ps = psA.tile([128, 512], F32, name="ps_s",
                                              tag="psA")
                                nc.tensor.matmul(
                                    ps[0:msz, 0:cw],
                                    lhsT=kt[:, ms:ms + msz],
                                    rhs=qt[:, cs:cs + cw],
                                    start=True, stop=True,
                                )
                                psS[hi].append(ps)
                        for hi in range(2):
                            pe = pep.tile([128, N], BF16, name="pe",
                                          tag=f"pe{hi}_{ms}")
                            nc.scalar.activation(pe[0:msz, 0:512],
                                                 psS[hi][0][0:msz, 0:512], EXP,
                                                 bias=0.0, scale=1.0)
                            nc.scalar.activation(pe[0:msz, 512:N],
                                                 psS[hi][1][0:msz, 0:65], EXP,
                                                 bias=0.0, scale=1.0)
                            Pe[hi].append(pe)

                    for hi in range(2):
                        h = 2 * hp + hi
                        psO = []
                        for (cs, cw) in NCH:
                            ps = psB.tile([128, 512], F32, name="ps_o",
                                          tag="psB")
                            for mi, (ms, msz) in enumerate(NT):
                                nc.tensor.matmul(
                                    ps[0:65, 0:cw],
                                    lhsT=V[mi][0:msz, h * 65:h * 65 + 65],
                                    rhs=Pe[hi][mi][0:msz, cs:cs + cw],
                                    start=(mi == 0), stop=(mi == 4),
                                )
                            psO.append(ps)
                        # evacuate psO (incl. sums row 64) so psum frees fast
                        outc = rp.tile([128, N], F32, name="outc", tag="outc")
                        nc.scalar.activation(outc[0:65, 0:512],
                                             psO[0][0:65, 0:512], COPY,
                                             bias=0.0, scale=1.0)
                        nc.scalar.activation(outc[0:65, 512:N],
                                             psO[1][0:65, 0:65], COPY,
                                             bias=0.0, scale=1.0)
                        recb = rp.tile([64, N], F32, name="recb", tag="recb")
                        # 1/s = exp(-ln(s)); ln+exp share one ACT table set so
                        # no table reloads, and this keeps recip off the DVE
                        nc.scalar.activation(recs[64:65, :], outc[64:65, 0:N],
                                             LN, bias=0.0, scale=1.0)
                        nc.scalar.activation(recs[64:65, :], recs[64:65, :],
                                             EXP, bias=0.0, scale=-1.0)
                        # broadcast partition 64 (lane 0 of quadrant 2) to
                        # partitions 0..63 via two 32-lane stream shuffles
                        bmask = [0] * 32
                        nc.vector.stream_shuffle(recb[0:32, :],
                                                 recs[64:96, :], bmask)
                        nc.vector.stream_shuffle(recb[32:64, :],
                                                 recs[64:96, :], bmask)
                        nc.vector.tensor_mul(at[hi * 64:hi * 64 + 64, 0:N],
                                             outc[0:64, 0:N], recb[0:64, :])

                # ---- proj + bias ----
                for (ns, nsz) in NT:
                    ob = obp.tile([128, C], F32, name="ob", tag="ob")
                    for (fs, fw) in FCH:
                        ps = psB.tile([128, 512], F32, name="ps_p", tag="psB")
                        for ct in range(6):
                            nc.tensor.matmul(
                                ps[0:nsz, 0:fw],
                                lhsT=ATT[ct][:, ns:ns + nsz],
                                rhs=wp[ct][:, fs:fs + fw],
                                start=(ct == 0), stop=(ct == 5),
                            )
                        nc.vector.tensor_add(ob[0:nsz, fs:fs + fw],
                                             ps[0:nsz, 0:fw],
                                             biasb[0:nsz, fs:fs + fw])
                    nc.sync.dma_start(out_d[b, ns:ns + nsz, :], ob[0:nsz, :])

    nc.compile()
    return nc


def _rope_perm():
    idx = []
    for h in range(H):
        base = h * D
        idx.extend(base + 2 * i for i in range(D // 2))      # evens
        idx.extend(base + 2 * i + 1 for i in range(D // 2))  # odds
    return np.array(idx)


def _prep_inputs(x, wqkv, wproj, bproj, freqs_cos, freqs_sin):
    perm = _rope_perm()
    wq = wqkv[0:C][perm] * 0.125
    wk = wqkv[C:2 * C][perm]
    wqkT = np.ascontiguousarray(
        np.concatenate([wq, wk], axis=0).T).astype(ml_dtypes.bfloat16)
    wvT = np.ascontiguousarray(wqkv[2 * C:].T).astype(ml_dtypes.bfloat16)
    wpT = np.ascontiguousarray(wproj.T).astype(ml_dtypes.bfloat16)
    cosT = np.ascontiguousarray(freqs_cos.T, dtype=np.float32)  # [32, 576]
    sinT = np.ascontiguousarray(freqs_sin.T, dtype=np.float32)
    c128 = np.concatenate([cosT] * 4, axis=0).astype(ml_dtypes.bfloat16)
    s128 = np.concatenate([-sinT, sinT, -sinT, sinT],
                          axis=0).astype(ml_dtypes.bfloat16)
    biasb = np.broadcast_to(bproj.astype(np.float32), (128, C)).copy()

    in_maps = []
    for core in range(NCORES):
        xs = x[core * BPC:(core + 1) * BPC]
        xT = np.ascontiguousarray(
            xs.transpose(0, 2, 1)).astype(ml_dtypes.bfloat16)
        in_maps.append({
            "xT": xT,
            "wqkT": wqkT,
            "wvT": wvT,
            "wpT": wpT,
            "c128": c128,
            "s128": s128,
            "biasb": biasb,
        })
    return in_maps


def kernel(x, wqkv, wproj, bproj, freqs_cos, freqs_sin, trace=False):
    global _CACHED_NC, LAST_RESULTS
    if _CACHED_NC is None:
        _CACHED_NC = _build_nc()
    in_maps = _prep_inputs(x, wqkv, wproj, bproj, freqs_cos, freqs_sin)
    res = run_bass_kernel_spmd(_CACHED_NC, in_maps,
                               core_ids=list(range(NCORES)), trace=trace)
    LAST_RESULTS = res
    out = np.concatenate([r["out"] for r in res.results], axis=0)
    return out.astype(np.float32)


# revision 26
# speedup vs baseline: 1.4269x; 1.4269x over previous
"""PerformerAttention (softmax attention + interleaved RoPE) on 8 trn2 cores.

Sharding: data-parallel over batch (16 batches -> 2 per core), weights replicated.

v5 design (all-bf16, transposed attention):
  qk^T = wqk^T.T @ x^T (bf16, fp32 accum) -> RoPE -> bf16 qkT tiles
  S^T = k^T q per head ([keys, queries] layout), head PAIRS row-packed on the
    128-row PE array (K=64 each -> tile_position (0,0)/(64,0))
  P^T = exp(S^T)  -- no max subtraction (logits bounded ~|8.7| for this data)
  out^T/sums = [V | 1]^T @ P^T  -- ones column interleaved in V gives softmax
    row-sums as psum row 64 for free
  softmax denominators: per-head sums rows are DMA'd onto partitions 0..11 of
    one tile; ONE Ln + ONE Exp(scale=-1) per batch computes 1/sums (2 ACT
    table loads per batch instead of 2 per head), then per-head 32-lane
    stream_shuffle broadcasts partition h to 64 lanes for the normalize mul.
  final = attnT.T @ wproj^T + bias

Host-side prep: shard x over batch, transpose + bf16-cast, permute wqkv q/k
rows for contiguous-block rope, fold 1/sqrt(D) into wq.
"""

import numpy as np
import ml_dtypes

import concourse.bass as bass
import concourse.mybir as mybir
import concourse.tile as tile
from concourse import bacc
from concourse.bass_utils import run_bass_kernel_spmd

F32 = mybir.dt.float32
BF16 = mybir.dt.bfloat16
COPY = mybir.ActivationFunctionType.Copy
EXP = mybir.ActivationFunctionType.Exp
LN = mybir.ActivationFunctionType.Ln

B, N, C, H, D = 16, 577, 768, 12, 64
NCORES = 8
BPC = B // NCORES  # batches per core
NM1 = N - 1  # 576 rope'd tokens
NT = [(0, 128), (128, 128), (256, 128), (384, 128), (512, 65)]  # token tiles
NCH = [(0, 512), (512, 65)]   # token free-dim chunks (psum bank = 512 fp32)
FCH = [(0, 512), (512, 256)]  # feature free-dim chunks
VW = 65 * H  # 780: V stored as 12 blocks of [v_h | ones]

_CACHED_NC = None
LAST_RESULTS = None  # test harness reads exec_time_ns off this


def _build_nc():
    nc = bacc.Bacc("TRN2", target_bir_lowering=False)

    xT_d = nc.dram_tensor("xT", [BPC, C, N], BF16, kind="ExternalInput")
    wqk_d = nc.dram_tensor("wqkT", [C, 2 * C], BF16, kind="ExternalInput")
    wv_d = nc.dram_tensor("wvT", [C, C], BF16, kind="ExternalInput")
    wp_d = nc.dram_tensor("wpT", [C, C], BF16, kind="ExternalInput")
    c_d = nc.dram_tensor("c128", [128, NM1], BF16, kind="ExternalInput")
    s_d = nc.dram_tensor("s128", [128, NM1], BF16, kind="ExternalInput")
    bias_d = nc.dram_tensor("biasb", [128, C], F32, kind="ExternalInput")
    out_d = nc.dram_tensor("out", [BPC, N, C], F32, kind="ExternalOutput")

    with tile.TileContext(nc) as tc:
        with (
            tc.tile_pool(name="const", bufs=1) as constp,
            tc.tile_pool(name="xp", bufs=2) as xp,
            tc.tile_pool(name="qkp", bufs=2) as qkp,
            tc.tile_pool(name="vp", bufs=2) as vp,
            tc.tile_pool(name="atp", bufs=2) as atp,
            tc.tile_pool(name="work", bufs=3) as work,
            tc.tile_pool(name="pep", bufs=2) as pep,
            tc.tile_pool(name="rp", bufs=2) as rp,
            tc.tile_pool(name="ocp", bufs=1) as ocp,
            tc.tile_pool(name="obp", bufs=3) as obp,
            tc.tile_pool(name="psA", bufs=2, space="PSUM") as psA,
            tc.tile_pool(name="psB", bufs=2, space="PSUM") as psB,
        ):
            # ---- constants / weights (once per core) ----
            wqk = []
            wv = []
            wp = []
            for ct in range(6):
                t = constp.tile([128, 2 * C], BF16, name=f"wqk{ct}", tag=f"wqk{ct}")
                nc.sync.dma_start(t, wqk_d[ct * 128:(ct + 1) * 128, :])
                wqk.append(t)
                t = constp.tile([128, C], BF16, name=f"wv{ct}", tag=f"wv{ct}")
                nc.sync.dma_start(t, wv_d[ct * 128:(ct + 1) * 128, :])
                wv.append(t)
                t = constp.tile([128, C], BF16, name=f"wp{ct}", tag=f"wp{ct}")
                nc.sync.dma_start(t, wp_d[ct * 128:(ct + 1) * 128, :])
                wp.append(t)
            c128 = constp.tile([128, NM1], BF16, name="c128", tag="c128")
            nc.sync.dma_start(c128, c_d[:, :])
            s128 = constp.tile([128, NM1], BF16, name="s128", tag="s128")
            nc.sync.dma_start(s128, s_d[:, :])
            biasb = constp.tile([128, C], F32, name="biasb", tag="biasb")
            nc.sync.dma_start(biasb, bias_d[:, :])

            for b in range(BPC):
                # per-head softmax sums collect here (partition h = head h);
                # rows 12..31 memset so the Ln/Exp + shuffle window is defined
                ssum = rp.tile([128, N], BF16, name="ssum", tag="ssum")
                nc.vector.memset(ssum[0:32, :], 1.0)

                # ---- load x^T bf16 tiles ----
                xT = []
                for ct in range(6):
                    t = xp.tile([128, N], BF16, name=f"xT{ct}", tag=f"xT{ct}")
                    nc.sync.dma_start(t, xT_d[b, ct * 128:(ct + 1) * 128, :])
                    xT.append(t)

                # ---- qk^T = wqk^T.T @ x^T  [f, n], then RoPE -> bf16 ----
                qkT = []
                for ft in range(12):
                    ps = psA.tile([128, 1024], F32, name="ps_qk", tag="psA")
                    for (cs, cw) in NCH:
                        for ct in range(6):
                            nc.tensor.matmul(
                                ps[:, cs:cs + cw],
                                lhsT=wqk[ct][:, ft * 128:(ft + 1) * 128],
                                rhs=xT[ct][:, cs:cs + cw],
                                start=(ct == 0), stop=(ct == 5),
                            )
                    wkt = work.tile([128, N], BF16, name="wkt", tag="wkt")
                    nc.scalar.activation(wkt[:, 0:N], ps[:, 0:N], COPY,
                                         bias=0.0, scale=1.0)
                    qk = qkp.tile([128, N], BF16, name=f"qkT{ft}",
                                  tag=f"qkT{ft}")
                    tsw = work.tile([128, NM1], BF16, name="tsw", tag="tsw")
                    rot = work.tile([128, NM1], BF16, name="rot", tag="rot")
                    # tsw = [to0; te0; to1; te1] (swap 32-row even/odd blocks)
                    nc.vector.tensor_copy(tsw[0:32, :], wkt[32:64, 1:N])
                    nc.vector.tensor_copy(tsw[32:64, :], wkt[0:32, 1:N])
                    nc.vector.tensor_copy(tsw[64:96, :], wkt[96:128, 1:N])
                    nc.vector.tensor_copy(tsw[96:128, :], wkt[64:96, 1:N])
                    nc.vector.tensor_mul(rot, wkt[:, 1:N], c128)
                    nc.vector.tensor_mul(tsw, tsw, s128)  # s128 = [-s;s;-s;s]
                    nc.vector.tensor_add(qk[:, 1:N], rot, tsw)
                    nc.vector.tensor_copy(qk[:, 0:1], wkt[:, 0:1])  # CLS
                    qkT.append(qk)

                # ---- V = x @ wv^T, interleaved [v_h | 1] blocks of 65 ----
                V = []
                for (ns, nsz) in NT:
                    nt = ns // 128
                    vt = vp.tile([128, VW], BF16, name=f"V{nt}", tag=f"V{nt}")
                    ones_ap = vt.rearrange("p (h d) -> p h d", d=65)[:, :, 64:65]
                    nc.vector.memset(ones_ap, 1.0)
                    ps = psB.tile([128, 1024], F32, name="ps_v", tag="psB")
                    for (fs, fw) in FCH:
                        for ct in range(6):
                            nc.tensor.matmul(
                                ps[0:nsz, fs:fs + fw],
                                lhsT=xT[ct][:, ns:ns + nsz],
                                rhs=wv[ct][:, fs:fs + fw],
                                start=(ct == 0), stop=(ct == 5),
                            )
                    dst = vt[0:nsz, :].rearrange(
                        "p (h d) -> p h d", d=65)[:, :, 0:64]
                    src = ps[0:nsz, 0:C].rearrange("p (h d) -> p h d", d=64)
                    nc.vector.tensor_copy(dst, src)
                    V.append(vt)

                # ---- attention, head pairs (row-packed S^T matmuls) ----
                ATT = []
                OUTC = []
                for hp in range(6):
                    at = atp.tile([128, N], BF16, name=f"attnT{hp}",
                                  tag=f"attnT{hp}")
                    ATT.append(at)
                    Pe = {0: [], 1: []}
                    for (ms, msz) in NT:
                        psS = {}
                        for hi in range(2):
                            psS[hi] = psA.tile([128, 1024], F32, name="ps_s",
                                               tag="psA")
                        for (cs, cw) in NCH:
                            for hi in range(2):
                                h = 2 * hp + hi
                                qt = qkT[h // 2][hi * 64:hi * 64 + 64, :]
                                kt = qkT[6 + h // 2][hi * 64:hi * 64 + 64, :]
                                nc.tensor.matmul(
                                    psS[hi][0:msz, cs:cs + cw],
                                    lhsT=kt[:, ms:ms + msz],
                                    rhs=qt[:, cs:cs + cw],
                                    start=True, stop=True,
                                )
                        for hi in range(2):
                            pe = pep.tile([128, N], BF16, name="pe",
                                          tag=f"pe{hi}_{ms}")
                            nc.scalar.activation(pe[0:msz, 0:N],
                                                 psS[hi][0:msz, 0:N], EXP,
                                                 bias=0.0, scale=1.0)
                            Pe[hi].append(pe)

                    for hi in range(2):
                        h = 2 * hp + hi
                        psO = psB.tile([128, 1024], F32, name="ps_o", tag="psB")
                        for (cs, cw) in NCH:
                            for mi, (ms, msz) in enumerate(NT):
                                nc.tensor.matmul(
                                    psO[0:65, cs:cs + cw],
                                    lhsT=V[mi][0:msz, h * 65:h * 65 + 65],
                                    rhs=Pe[hi][mi][0:msz, cs:cs + cw],
                                    start=(mi == 0), stop=(mi == 4),
                                )
                        # evacuate psO (rows 0..64 = out, row 64 = sums)
                        outc = ocp.tile([128, N], BF16, name=f"outc{h}",
                                        tag=f"outc{h}")
                        nc.vector.tensor_copy(outc[0:65, 0:N], psO[0:65, 0:N])
                        OUTC.append(outc)
                        # stage this head's sums row onto partition h of ssum
                        nc.sync.dma_start(ssum[h:h + 1, 0:N],
                                          outc[64:65, 0:N])

                # ---- batched softmax denominators: 1/s = exp(-ln(s)) ----
                # one Ln + one Exp per batch -> only 2 ACT table reloads
                nc.scalar.activation(ssum[0:32, :], ssum[0:32, :], LN,
                                     bias=0.0, scale=1.0)
                nc.scalar.activation(ssum[0:32, :], ssum[0:32, :], EXP,
                                     bias=0.0, scale=-1.0)

                # ---- normalize: attnT[h] = outc[h] * (1/sums_h) ----
                for h in range(12):
                    recb = rp.tile([64, N], BF16, name="recb", tag="recb")
                    bmask = [h] * 32
                    nc.vector.stream_shuffle(recb[0:32, :], ssum[0:32, :],
                                             bmask)
                    nc.vector.stream_shuffle(recb[32:64, :], ssum[0:32, :],
                                             bmask)
                    nc.vector.tensor_mul(
                        ATT[h // 2][(h % 2) * 64:(h % 2) * 64 + 64, 0:N],
                        OUTC[h][0:64, 0:N], recb[0:64, :])

                # ---- proj + bias ----
                for (ns, nsz) in NT:
                    ps = psB.tile([128, 1024], F32, name="ps_p", tag="psB")
                    for (fs, fw) in FCH:
                        for ct in range(6):
                            nc.tensor.matmul(
                                ps[0:nsz, fs:fs + fw],
                                lhsT=ATT[ct][:, ns:ns + nsz],
                                rhs=wp[ct][:, fs:fs + fw],
                                start=(ct == 0), stop=(ct == 5),
                            )
                    ob = obp.tile([128, C], F32, name="ob", tag="ob")
                    nc.vector.tensor_add(ob[0:nsz, :], ps[0:nsz, 0:C],
                                         biasb[0:nsz, :])
                    nc.sync.dma_start(out_d[b, ns:ns + nsz, :], ob[0:nsz, :])

    nc.compile()
    return nc


def _rope_perm():
    idx = []
    for h in range(H):
        base = h * D
        idx.extend(base + 2 * i for i in range(D // 2))      # evens
        idx.extend(base + 2 * i + 1 for i in range(D // 2))  # odds
    return np.array(idx)


def _prep_inputs(x, wqkv, wproj, bproj, freqs_cos, freqs_sin):
    perm = _rope_perm()
    wq = wqkv[0:C][perm] * 0.125
    wk = wqkv[C:2 * C][perm]
    wqkT = np.ascontiguousarray(
        np.concatenate([wq, wk], axis=0).T).astype(ml_dtypes.bfloat16)
    wvT = np.ascontiguousarray(wqkv[2 * C:].T).astype(ml_dtypes.bfloat16)
    wpT = np.ascontiguousarray(wproj.T).astype(ml_dtypes.bfloat16)
    cosT = np.ascontiguousarray(freqs_cos.T, dtype=np.float32)  # [32, 576]
    sinT = np.ascontiguousarray(freqs_sin.T, dtype=np.float32)
    c128 = np.concatenate([cosT] * 4, axis=0).astype(ml_dtypes.bfloat16)
    s128 = np.concatenate([-sinT, sinT, -sinT, sinT],
                          axis=0).astype(ml_dtypes.bfloat16)
    biasb = np.broadcast_to(bproj.astype(np.float32), (128, C)).copy()

    in_maps = []
    for core in range(NCORES):
        xs = x[core * BPC:(core + 1) * BPC]
        xT = np.ascontiguousarray(
            xs.transpose(0, 2, 1)).astype(ml_dtypes.bfloat16)
        in_maps.append({
            "xT": xT,
            "wqkT": wqkT,
            "wvT": wvT,
            "wpT": wpT,
            "c128": c128,
            "s128": s128,
            "biasb": biasb,
        })
    return in_maps


def kernel(x, wqkv, wproj, bproj, freqs_cos, freqs_sin, trace=False):
    global _CACHED_NC, LAST_RESULTS
    if _CACHED_NC is None:
        _CACHED_NC = _build_nc()
    in_maps = _prep_inputs(x, wqkv, wproj, bproj, freqs_cos, freqs_sin)
    res = run_bass_kernel_spmd(_CACHED_NC, in_maps,
                               core_ids=list(range(NCORES)), trace=trace)
    LAST_RESULTS = res
    out = np.concatenate([r["out"] for r in res.results], axis=0)
    return out.astype(np.float32)
